# revision 7
# baseline (speedup 1.0000x reference)
"""Adaptive average pooling (16,250,250,256) -> (16,7,7,256), NHWC, f32.

Strategy: data-parallel over batch (2 images per NeuronCore, 8 cores).
Per core, both pooling axes are done on the TensorEngine:
  - SBUF layout puts W on partitions (125 partitions, 2 w per partition
    as separate free-dim columns), so the column (W) pooling is a matmul
    with a [125, 7] binning weight matrix (values 1/count_w[j]).
  - The row (H) pooling is PSUM accumulation: every h row's matmul
    accumulates into the PSUM slab of its row-bin (7 slabs, one bank each).
  - Epilogue: ScalarE copies each slab PSUM->SBUF scaled by 1/count_h[i].
Matmuls run in float32r (TF32-like, 1 cycle/row at N>=256) so the PE
streams at ~300 Ge/s while DMA (~360 GB/s/core) is the bottleneck.
"""

import sys

for _p in ("/opt/trn_rl_repo",):
    if _p not in sys.path:
        sys.path.insert(0, _p)

import numpy as np

from concourse import bacc, mybir, tile
from concourse.bass_utils import run_bass_kernel_spmd

B, H, W, C = 16, 250, 250, 256
OUT_H = OUT_W = 7
NCORES = 8
BPC = B // NCORES  # batches per core

NH = 25   # h rows per DMA block (25 KB/partition/tile)
P = 125   # partitions: w pairs (w = 2p + parity)


def _bin_edges(in_size, out_size):
    scale = np.float32(in_size / out_size)
    idx = np.arange(out_size, dtype=np.float32)
    starts = (idx * scale).astype(np.int32)
    ends = np.ceil((idx + 1.0) * scale).astype(np.int32)
    return starts, ends


SX, EX = _bin_edges(H, OUT_H)
SY, EY = _bin_edges(W, OUT_W)
CH = EX - SX
CW = EY - SY

_NC_CACHE = []


def _build():
    nc = bacc.Bacc("TRN2", target_bir_lowering=False, debug=False,
                   num_devices=NCORES)
    f32 = mybir.dt.float32
    f32r = mybir.dt.float32r
    x = nc.dram_tensor("x", [BPC, H, W, C], f32r, kind="ExternalInput").ap()
    q = nc.dram_tensor("q", [2, P, OUT_W], f32r, kind="ExternalInput").ap()
    out = nc.dram_tensor("out", [BPC, OUT_H, OUT_W, C], f32,
                         kind="ExternalOutput").ap()

    with tile.TileContext(nc) as tc:
        with tc.tile_pool(name="const", bufs=1) as cpool, \
             tc.tile_pool(name="xp", bufs=3) as xpool, \
             tc.tile_pool(name="op", bufs=2) as opool, \
             tc.tile_pool(name="ps", bufs=1, space="PSUM") as pspool:
            # Binning weights: qts[par][p, j] = 1/CW[j] if (2p+par) in w-bin j
            qts = []
            for par in range(2):
                qt = cpool.tile([P, OUT_W], f32r, name=f"q{par}")
                nc.sync.dma_start(qt[:], q[par])
                qts.append(qt)

            for b in range(BPC):
                ps = [pspool.tile([OUT_H, C], f32, tag=f"ps{i}",
                                  name=f"ps{i}b{b}") for i in range(OUT_H)]
                for hb in range(H // NH):
                    h0 = hb * NH
                    xt = xpool.tile([P, NH * 2 * C], f32r, tag="x",
                                    name=f"x{b}_{hb}")
                    xt3 = xt.rearrange("p (h w c) -> p h w c",
                                       h=NH, w=2, c=C)
                    src = x[b, h0:h0 + NH, :, :].rearrange(
                        "h (p w) c -> p h w c", w=2)
                    nc.sync.dma_start(xt3, src)
                    for ih in range(NH):
                        h = h0 + ih
                        bins = [i for i in range(OUT_H)
                                if SX[i] <= h < EX[i]]
                        for par in range(2):
                            rhs = xt3[:, ih, par, :]
                            lhsT = qts[par][:]
                            for i in bins:
                                nc.tensor.matmul(
                                    ps[i][:], lhsT, rhs,
                                    start=(h == SX[i] and par == 0),
                                    stop=(h == EX[i] - 1 and par == 1),
                                )
                osb = opool.tile([OUT_H, OUT_H * C], f32, tag="osb",
                                 name=f"osb{b}")
                for i in range(OUT_H):
                    nc.scalar.mul(osb[:, i * C:(i + 1) * C], ps[i][:],
                                  1.0 / float(CH[i]))
                nc.sync.dma_start(
                    out[b].rearrange("i j c -> j i c"),
                    osb.rearrange("j (i c) -> j i c", c=C))

    nc.compile()
    return nc


def _get_nc():
    if not _NC_CACHE:
        _NC_CACHE.append(_build())
    return _NC_CACHE[0]


def _q_np():
    qv = np.zeros((2, P, OUT_W), dtype=np.float32)
    for par in range(2):
        for p in range(P):
            w = 2 * p + par
            for j in range(OUT_W):
                if SY[j] <= w < EY[j]:
                    qv[par, p, j] = 1.0 / float(CW[j])
    return qv


def run(x: np.ndarray, **spmd_kwargs):
    x = np.ascontiguousarray(x, dtype=np.float32)
    assert x.shape == (B, H, W, C), x.shape
    nc = _get_nc()
    qv = _q_np()
    in_maps = [{"x": x[i * BPC:(i + 1) * BPC], "q": qv}
               for i in range(NCORES)]
    res = run_bass_kernel_spmd(nc, in_maps, core_ids=list(range(NCORES)),
                               **spmd_kwargs)
    out = np.concatenate([res.results[i]["out"] for i in range(NCORES)],
                         axis=0)
    return out, res


def kernel(x: np.ndarray) -> np.ndarray:
    out, _ = run(x)
    return out
